# revision 18
# baseline (speedup 1.0000x reference)
"""GQA attention kernel for Trainium2, 8 NeuronCores.

Sharding: core c -> (batch = c // 4, head-group g = c % 4).
Each core handles one batch and 8 contiguous Q heads (= 2 KV heads),
computes its slice of Q/K/V projections, RoPE, causal attention, and a
partial output projection (rows g*512:(g+1)*512 of Wo). Host sums the 4
partials per batch.

Layout: projections are computed transposed (W^T-stationary matmuls on
x^T) producing Q^T/K^T ([feature, token]), the layout the scores matmul
needs. V is projected directly in [token, feature] layout (x^T tile as
the stationary operand) so no PE transposes are needed. Scores are
computed transposed (S^T[j,i], keys on partitions) in 1024-query
blocks so the context matmul consumes exp(S^T) with V as the stationary
operand; a ones column appended to V yields softmax denominators in the
same PSUM accumulation.

Scheduling is built around the PE HAM clock gate (PE runs at 1.2 GHz
until it sees a ~3.4us window of gapless activity, then 2.4 GHz):
 - attention is software-pipelined per head with a one-j-step lag
   between scores and the context matmul consuming its exp;
 - the projection tail is interleaved between attention heads of the
   first query half, and output-projection token-tiles (16 dense
   back-to-back matmuls each) are injected between heads of the second
   half, so the PE always has long wait-free runs that keep the clock
   warm;
 - bulk inputs are loaded with single rearranged DMAs (dma_start issue
   costs ~0.6us each on the sync queue), and small rope/output DMAs go
   through the gpsimd queue.
"""

import sys
import math

for _p in ("/opt/trn_rl_repo",):
    if _p not in sys.path:
        sys.path.append(_p)

import numpy as np
import ml_dtypes

import concourse.bass as bass
from concourse import bacc
import concourse.mybir as mybir
import concourse.tile as tile
from concourse.bass_utils import run_bass_kernel_spmd

BF16 = mybir.dt.bfloat16
F32 = mybir.dt.float32

B, S, D = 2, 2048, 2048
NH, NKV, HD = 32, 8, 64
GROUP = NH // NKV          # 4 q heads per kv head
NCORES = 8
CPB = NCORES // B          # 4 cores per batch
HPC = NH // CPB            # 8 q heads per core
KVPC = NKV // CPB          # 2 kv heads per core
QW = HPC * HD              # 512 projected q cols per core
KW = KVPC * HD             # 128 projected kv cols per core

NT = S // 128              # 16 seq tiles of 128
NBL = S // 512             # 4 seq blocks of 512
KT = D // 128              # 16 contraction tiles
QF = QW // 128             # 4 row-tiles of Q^T

_nc_cache = None
_DEBUG = False


def _build():
    nc = bacc.Bacc()
    xT = nc.dram_tensor("xT", [D, S], BF16, kind="ExternalInput")
    wq = nc.dram_tensor("wq", [D, QW], BF16, kind="ExternalInput")
    wk = nc.dram_tensor("wk", [D, KW], BF16, kind="ExternalInput")
    wv = nc.dram_tensor("wv", [D, KW], BF16, kind="ExternalInput")
    wo = nc.dram_tensor("wo", [QW, D], BF16, kind="ExternalInput")
    cos2 = nc.dram_tensor("cos2", [128, S], BF16, kind="ExternalInput")
    sinm = nc.dram_tensor("sinm", [128, S], BF16, kind="ExternalInput")
    msk = nc.dram_tensor("msk", [128, 128], BF16, kind="ExternalInput")
    out = nc.dram_tensor("out", [S, D], BF16, kind="ExternalOutput")
    if _DEBUG:
        qdbg = nc.dram_tensor("qdbg", [QF, 128, S], BF16, kind="ExternalOutput")
        kdbg = nc.dram_tensor("kdbg", [128, S], BF16, kind="ExternalOutput")
        vdbg = nc.dram_tensor("vdbg", [KVPC, 128, NT * 65], BF16, kind="ExternalOutput")
        cdbg = nc.dram_tensor("cdbg", [QF, 128, S], BF16, kind="ExternalOutput")

    Exp = mybir.ActivationFunctionType.Exp

    with tile.TileContext(nc) as tc:
        with (
            tc.tile_pool(name="persist", bufs=1) as pp,
            tc.tile_pool(name="psum", bufs=1, space="PSUM") as psp,
            tc.tile_pool(name="attn", bufs=4) as ap,
            tc.tile_pool(name="smal", bufs=1) as sp,
            tc.tile_pool(name="ostg", bufs=2) as op,
        ):
            # ---- persistent tiles ----
            Qb = [pp.tile([128, S], BF16, name=f"qb{f}", tag=f"qb{f}") for f in range(QF)]
            Kb = pp.tile([128, S], BF16, name="kb", tag="kb")
            Vaug = [pp.tile([128, NT, 65], BF16, name=f"vaug{k}", tag=f"vaug{k}") for k in range(KVPC)]
            ctxT = [pp.tile([128, S], BF16, name=f"ctxt{f}", tag=f"ctxt{f}") for f in range(QF)]
            trit = pp.tile([128, 128], BF16, name="trit", tag="trit")

            for k in range(KVPC):
                nc.gpsimd.memset(Vaug[k][:, :, 64:65], 1.0)

            def st_tile():
                return psp.tile([128, 512], F32, name="st", tag="st",
                                bufs=2, padded_shape=[128, 1024])

            def st2_tile():
                return psp.tile([128, 1024], F32, name="st2", tag="st",
                                bufs=2)

            def cp_tile(i):
                return psp.tile([128, 1024], F32, name=f"cp{i}",
                                tag=f"cp{i % 2}", bufs=1)

            # ---- attention block (1024-query half ib2 of head hl) ----
            def attn_block(hl, ib2, inject=None, inject_j=8):
                kv = hl // GROUP
                fq = hl % GROUP
                qr = kv * 64
                q0 = ib2 * 1024
                jmax = 8 * ib2 + 7
                cp = cp_tile(hl % 2)

                def do_scores(j):
                    jb = j * 128
                    c0 = max(0, jb - q0)
                    st = st2_tile()
                    for lo in (0, 512):
                        hi = lo + 512
                        if hi <= c0:
                            continue
                        l0 = max(lo, c0)
                        nc.tensor.matmul(
                            st[:, l0:hi],
                            Kb[kv * 64:(kv + 1) * 64, jb:jb + 128],
                            Qb[fq][qr:qr + 64, q0 + l0:q0 + hi],
                            start=True, stop=True)
                    pt = ap.tile([128, 1024], BF16, name="pt", tag="pt")
                    nc.scalar.activation(pt[:, c0:1024], st[:, c0:1024],
                                         Exp, scale=0.125)
                    if 0 <= jb - q0 < 1024:
                        # diagonal tile: only the 128-wide band at the
                        # causal boundary needs the triangle mask
                        nc.vector.tensor_mul(pt[:, c0:c0 + 128],
                                             pt[:, c0:c0 + 128],
                                             trit[:])
                    return (j, c0, pt)

                def do_ctx(item):
                    (j, c0, pt) = item
                    for lo in (0, 512):
                        hi = lo + 512
                        if hi <= c0:
                            continue
                        l0 = max(lo, c0)
                        nc.tensor.matmul(
                            cp[0:65, l0:hi], Vaug[kv][:, j, :],
                            pt[:, l0:hi],
                            start=(j == 0), stop=(j == jmax),
                            skip_group_check=True)
                    if j == jmax:
                        norm_ctx()

                def norm_ctx():
                    # denominators sit in row 64 of cp; normalize the 64 ctx
                    # rows into ctxT, freeing the bank. (denom must bounce
                    # via SBUF: custom-DVE ops read garbage from PSUM on hw)
                    dn = sp.tile([1, 1024], F32, name="dn", tag="dn")
                    nc.vector.tensor_copy(dn[0:1, :], cp[64:65, :])
                    rc = sp.tile([1, 1024], F32, name="rc", tag="rc")
                    nc.vector.reciprocal_approx_fast(rc[0:1, :], dn[0:1, :])
                    bc = sp.tile([64, 1024], F32, name="bc", tag="bc")
                    nc.gpsimd.partition_broadcast(bc[0:64, :], rc[0:1, :])
                    nc.vector.tensor_mul(
                        ctxT[fq][qr:qr + 64, q0:q0 + 1024],
                        cp[0:64, :], bc[0:64, :])

                prev = None
                for j in range(jmax + 1):
                    cur = do_scores(j)
                    if prev is not None:
                        do_ctx(prev)
                    if inject is not None and j == inject_j:
                        inject()
                    prev = cur
                do_ctx(prev)

            # ==== phase 1+2: load x/weights, projections + rope ====
            # (the projection tail is interleaved into attention 3a below)
            with (
                tc.tile_pool(name="proj", bufs=1) as jp,
                tc.tile_pool(name="rope", bufs=2) as rp,
            ):
                xb = jp.tile([128, KT, S], BF16, name="xb", tag="xb")
                wqb = jp.tile([128, KT, QW], BF16, name="wqb", tag="wqb")
                wkb = jp.tile([128, KT, KW], BF16, name="wkb", tag="wkb")
                wvb = jp.tile([128, KT, KW], BF16, name="wvb", tag="wvb")
                cos2t = jp.tile([128, S], BF16, name="cos2t", tag="cos2t")
                sinmt = jp.tile([128, S], BF16, name="sinmt", tag="sinmt")

                # single rearranged DMAs, ordered to match compute
                nc.sync.dma_start(
                    wkb[:], wk[:, :].rearrange("(k p) w -> p k w", p=128))
                nc.sync.dma_start(
                    xb[:, :, 0:512],
                    xT[:, 0:512].rearrange("(k p) c -> p k c", p=128))
                nc.sync.dma_start(
                    wvb[:], wv[:, :].rearrange("(k p) w -> p k w", p=128))
                nc.sync.dma_start(
                    xb[:, :, 512:1024],
                    xT[:, 512:1024].rearrange("(k p) c -> p k c", p=128))
                nc.sync.dma_start(cos2t[:], cos2[:, :])
                nc.sync.dma_start(sinmt[:], sinm[:, :])
                nc.sync.dma_start(
                    wqb[:], wq[:, :].rearrange("(k p) w -> p k w", p=128))
                nc.scalar.dma_start(
                    xb[:, :, 1024:1536],
                    xT[:, 1024:1536].rearrange("(k p) c -> p k c", p=128))
                nc.scalar.dma_start(
                    xb[:, :, 1536:S],
                    xT[:, 1536:S].rearrange("(k p) c -> p k c", p=128))
                nc.scalar.dma_start(trit[:], msk[:, :])

                def rope_store(ps, dst, tcol):
                    # ps: psum [128, 512] f32 holding raw Q^T/K^T rows.
                    # dst[:, tcol:tcol+512] <- rope(ps) in bf16.
                    qf = rp.tile([128, 512], F32, name="ropecp", tag="ropecp")
                    nc.scalar.copy(qf[:], ps[:])
                    rot = rp.tile([128, 512], F32, name="roperot", tag="roperot")
                    for base in (0, 64):
                        nc.gpsimd.dma_start(rot[base:base + 32, :],
                                            qf[base + 32:base + 64, :])
                        nc.gpsimd.dma_start(rot[base + 32:base + 64, :],
                                            qf[base:base + 32, :])
                    a = rp.tile([128, 512], F32, name="ropea", tag="ropea")
                    b = rp.tile([128, 512], F32, name="ropeb", tag="ropeb")
                    nc.vector.tensor_mul(a[:], qf[:], cos2t[:, tcol:tcol + 512])
                    nc.vector.tensor_mul(b[:], rot[:], sinmt[:, tcol:tcol + 512])
                    nc.vector.tensor_add(dst[:, tcol:tcol + 512], a[:], b[:])

                def proj_k(t):
                    ps = st_tile()
                    for k in range(KT):
                        nc.tensor.matmul(
                            ps[:], wkb[:, k, :], xb[:, k, t * 512:(t + 1) * 512],
                            start=(k == 0), stop=(k == KT - 1))
                    rope_store(ps, Kb, t * 512)

                def proj_q(f, t):
                    ps = st_tile()
                    for k in range(KT):
                        nc.tensor.matmul(
                            ps[:], wqb[:, k, f * 128:(f + 1) * 128],
                            xb[:, k, t * 512:(t + 1) * 512],
                            start=(k == 0), stop=(k == KT - 1))
                    rope_store(ps, Qb[f], t * 512)

                def proj_v(tt):
                    # V directly in [token, feature] layout: x^T tile is the
                    # stationary operand, wv streams. out [128 tok, 128 feat].
                    ps = psp.tile([128, 128], F32, name="vp", tag="st", bufs=2,
                                  padded_shape=[128, 1024])
                    for k in range(KT):
                        nc.tensor.matmul(
                            ps[:], xb[:, k, tt * 128:(tt + 1) * 128],
                            wvb[:, k, 0:KW],
                            start=(k == 0), stop=(k == KT - 1))
                    for kv in range(KVPC):
                        nc.vector.tensor_copy(Vaug[kv][:, tt, 0:64],
                                              ps[:, kv * 64:(kv + 1) * 64])

                # everything attention 3a needs (keys/queries 0:1024):
                proj_k(0)
                for tt in range(0, 4):
                    proj_v(tt)
                proj_k(1)
                for tt in range(4, 8):
                    proj_v(tt)
                for f in range(QF):
                    proj_q(f, 0)
                for f in range(QF):
                    proj_q(f, 1)

                # remaining projection work, interleaved between 3a heads
                units = ([lambda t=t: proj_k(t) for t in (2, 3)]
                         + [lambda tt=tt: proj_v(tt) for tt in range(8, 16)]
                         + [lambda f=f, t=t: proj_q(f, t)
                            for t in (2, 3) for f in range(QF)])

                # ==== phase 3a: first query half, proj tail interleaved
                # (one unit injected mid-block, one between blocks) ====
                ui = [0]

                def next_unit():
                    if ui[0] < len(units):
                        units[ui[0]]()
                        ui[0] += 1

                for hl in range(HPC):
                    attn_block(hl, 0, inject=next_unit, inject_j=4)
                    next_unit()
                while ui[0] < len(units):
                    next_unit()

            # ==== phase 3b + 4: second half + output projection ====
            with tc.tile_pool(name="wout", bufs=1) as wp:
                wot = [wp.tile([128, D], BF16, name=f"wot{c}", tag=f"wot{c}")
                       for c in range(QF)]
                for c in range(QF):
                    nc.sync.dma_start(wot[c][:], wo[c * 128:(c + 1) * 128, :])

                def outproj_tile(t, tagid):
                    # one full output token-tile: 16 dense matmuls with no
                    # cross-engine deps — a long wait-free PE run that fires
                    # the HAM warm-up when injected inside attention.
                    ob = op.tile([128, D], BF16, name="ob", tag="ob")
                    ps2 = psp.tile([128, 2, 512], F32, name="ops",
                                   tag=f"cp{tagid}", bufs=1)
                    for o in range(NBL):
                        h = o % 2
                        for c in range(QF):
                            nc.tensor.matmul(
                                ps2[:, h, :],
                                ctxT[c][:, t * 128:(t + 1) * 128],
                                wot[c][:, o * 512:(o + 1) * 512],
                                start=(c == 0), stop=(c == QF - 1))
                        nc.vector.tensor_copy(ob[:, o * 512:(o + 1) * 512],
                                              ps2[:, h, :])
                    nc.gpsimd.dma_start(out[t * 128:(t + 1) * 128, :], ob[:])

                # 2-head lockstep: both heads of a pair advance j
                # together so the ACT queue never head-of-line blocks on a
                # just-issued scores matmul; outproj tiles are injected in
                # dense 32-matmul bursts between pairs.
                def attn_pair(hA, hB):
                    blkA = attn_steps(hA, 1)
                    blkB = attn_steps(hB, 1)
                    for a, b in zip(blkA, blkB):
                        a()
                        b()

                def attn_steps(hl, ib2):
                    kv = hl // GROUP
                    fq = hl % GROUP
                    qr = kv * 64
                    q0 = ib2 * 1024
                    jmax = 8 * ib2 + 7
                    state = {}

                    def scores_step(j):
                        def f():
                            state[j] = _scores(hl, kv, fq, qr, q0, j)
                        return f

                    def ctx_step(j):
                        def f():
                            _ctx(hl, kv, state.pop(j), jmax,
                                 state["cp"], fq, qr, q0)
                        return f

                    def open_cp():
                        state["cp"] = cp_tile(hl % 2)

                    steps = []

                    def first():
                        open_cp()
                        scores_step(0)()
                    steps.append(first)
                    for j in range(1, jmax + 1):
                        def f(j=j):
                            scores_step(j)()
                            ctx_step(j - 1)()
                        steps.append(f)
                    steps.append(ctx_step(jmax))
                    return steps

                def _scores(hl, kv, fq, qr, q0, j):
                    jb = j * 128
                    c0 = max(0, jb - q0)
                    st = st2_tile()
                    for lo in (0, 512):
                        hi = lo + 512
                        if hi <= c0:
                            continue
                        l0 = max(lo, c0)
                        nc.tensor.matmul(
                            st[:, l0:hi],
                            Kb[kv * 64:(kv + 1) * 64, jb:jb + 128],
                            Qb[fq][qr:qr + 64, q0 + l0:q0 + hi],
                            start=True, stop=True)
                    pt = ap.tile([128, 1024], BF16, name="pt", tag="pt")
                    nc.scalar.activation(pt[:, c0:1024], st[:, c0:1024],
                                         Exp, scale=0.125)
                    if 0 <= jb - q0 < 1024:
                        nc.vector.tensor_mul(pt[:, c0:c0 + 128],
                                             pt[:, c0:c0 + 128],
                                             trit[:])
                    return (j, c0, pt)

                def _ctx(hl, kv, item, jmax, cp, fq, qr, q0):
                    (j, c0, pt) = item
                    for lo in (0, 512):
                        hi = lo + 512
                        if hi <= c0:
                            continue
                        l0 = max(lo, c0)
                        nc.tensor.matmul(
                            cp[0:65, l0:hi], Vaug[kv][:, j, :],
                            pt[:, l0:hi],
                            start=(j == 0), stop=(j == jmax),
                            skip_group_check=True)
                    if j == jmax:
                        dn = sp.tile([1, 1024], F32, name="dn", tag="dn")
                        nc.vector.tensor_copy(dn[0:1, :], cp[64:65, :])
                        rc = sp.tile([1, 1024], F32, name="rc", tag="rc")
                        nc.vector.reciprocal_approx_fast(rc[0:1, :], dn[0:1, :])
                        bc = sp.tile([64, 1024], F32, name="bc", tag="bc")
                        nc.gpsimd.partition_broadcast(bc[0:64, :], rc[0:1, :])
                        nc.vector.tensor_mul(
                            ctxT[fq][qr:qr + 64, q0:q0 + 1024],
                            cp[0:64, :], bc[0:64, :])

                for p in range(HPC // 2):
                    attn_pair(2 * p, 2 * p + 1)
                    outproj_tile(2 * p, 0)
                    outproj_tile(2 * p + 1, 1)

                # ==== phase 4: remaining out token-tiles, stationary-reuse
                # loop order (ldweights once per contraction tile) ====
                for t in range(8, NT):
                    ob = op.tile([128, D], BF16, name="ob", tag="ob")
                    pst = [psp.tile([128, 512], F32, name="ops4", tag=tag,
                                    bufs=bf, padded_shape=[128, 1024])
                           for tag, bf in (("st", 2), ("st", 2),
                                           ("cp0", 1), ("cp1", 1))]
                    for c in range(QF):
                        for o in range(NBL):
                            nc.tensor.matmul(
                                pst[o][:],
                                ctxT[c][:, t * 128:(t + 1) * 128],
                                wot[c][:, o * 512:(o + 1) * 512],
                                start=(c == 0), stop=(c == QF - 1))
                    for o in range(NBL):
                        nc.vector.tensor_copy(ob[:, o * 512:(o + 1) * 512],
                                              pst[o][:])
                    nc.gpsimd.dma_start(out[t * 128:(t + 1) * 128, :], ob[:])

                if _DEBUG:
                    for f in range(QF):
                        nc.sync.dma_start(qdbg[f, :, :], Qb[f][:])
                        nc.sync.dma_start(cdbg[f, :, :], ctxT[f][:])
                    nc.sync.dma_start(kdbg[:, :], Kb[:])
                    for kv in range(KVPC):
                        nc.sync.dma_start(vdbg[kv, :, :],
                                          Vaug[kv][:, :, :])

    nc.finalize()
    return nc


def _get_nc():
    global _nc_cache
    if _nc_cache is None:
        _nc_cache = _build()
    return _nc_cache


def _prep_inputs(x, cos, sin, Wq, Wk, Wv, Wo):
    bf = ml_dtypes.bfloat16
    cosT = np.ascontiguousarray(cos.T.astype(np.float32))          # [64, S]
    sinT = sin.T.astype(np.float32)
    sinm64 = np.concatenate([-sinT[:32], sinT[32:]], axis=0)       # [64, S]
    cos2 = np.ascontiguousarray(np.concatenate([cosT, cosT], 0)).astype(bf)
    sinm = np.ascontiguousarray(np.concatenate([sinm64, sinm64], 0)).astype(bf)
    msk = (np.arange(128)[:, None] <= np.arange(128)[None, :]).astype(bf)

    # head permutation: Q^T tile f holds local heads (f, f+4) so that the
    # kv0/kv1 row base of K matches the q row base (PE base-partition rule)
    perm = [0, 4, 1, 5, 2, 6, 3, 7]
    colperm = np.concatenate(
        [np.arange(HD) + p * HD for p in perm])          # [QW]
    in_maps = []
    for c in range(NCORES):
        b, g = c // CPB, c % CPB
        xTb = np.ascontiguousarray(x[b].T.astype(bf))
        wq_g = Wq[:, g * QW:(g + 1) * QW][:, colperm]
        wo_g = Wo[g * QW:(g + 1) * QW, :][colperm, :]
        in_maps.append({
            "xT": xTb,
            "wq": np.ascontiguousarray(wq_g.astype(bf)),
            "wk": np.ascontiguousarray(Wk[:, g * KW:(g + 1) * KW].astype(bf)),
            "wv": np.ascontiguousarray(Wv[:, g * KW:(g + 1) * KW].astype(bf)),
            "wo": np.ascontiguousarray(wo_g.astype(bf)),
            "cos2": cos2,
            "sinm": sinm,
            "msk": msk,
        })
    return in_maps


def kernel(x, mask, cos, sin, Wq, Wk, Wv, Wo, _trace=False, **kw):
    x = np.asarray(x, dtype=np.float32)
    in_maps = _prep_inputs(x, np.asarray(cos), np.asarray(sin),
                           np.asarray(Wq), np.asarray(Wk),
                           np.asarray(Wv), np.asarray(Wo))
    nc = _get_nc()
    res = run_bass_kernel_spmd(nc, in_maps, core_ids=list(range(NCORES)),
                               trace=_trace, **kw)
    parts = [np.asarray(r["out"], dtype=np.float32) for r in res.results]
    full = np.stack([
        sum(parts[b * CPB + g] for g in range(CPB)) for b in range(B)
    ]).astype(np.float32)
    if _trace:
        kernel.last_result = res
    return full


# revision 19
# speedup vs baseline: 1.1682x; 1.1682x over previous
"""GQA attention kernel for Trainium2, 8 NeuronCores.

Sharding: core c -> (batch = c // 4, head-group g = c % 4).
Each core handles one batch and 8 contiguous Q heads (= 2 KV heads),
computes its slice of Q/K/V projections, RoPE, causal attention, and a
partial output projection (rows g*512:(g+1)*512 of Wo). Host sums the 4
partials per batch.

Layout: projections are computed transposed (W^T-stationary matmuls on
x^T) producing Q^T/K^T ([feature, token]), the layout the scores matmul
needs. V is projected directly in [token, feature] layout (x^T tile as
the stationary operand) so no PE transposes are needed. Scores are
computed transposed (S^T[j,i], keys on partitions) in 1024-query
blocks so the context matmul consumes exp(S^T) with V as the stationary
operand; a ones column appended to V yields softmax denominators in the
same PSUM accumulation.

Scheduling is built around the PE HAM clock gate (PE runs at 1.2 GHz
until it sees a ~3.4us window of gapless activity, then 2.4 GHz):
 - attention is software-pipelined per head with a one-j-step lag
   between scores and the context matmul consuming its exp;
 - the projection tail is interleaved between attention heads of the
   first query half, and output-projection token-tiles (16 dense
   back-to-back matmuls each) are injected between heads of the second
   half, so the PE always has long wait-free runs that keep the clock
   warm;
 - bulk inputs are loaded with single rearranged DMAs (dma_start issue
   costs ~0.6us each on the sync queue), and small rope/output DMAs go
   through the gpsimd queue.
"""

import sys
import math

for _p in ("/opt/trn_rl_repo",):
    if _p not in sys.path:
        sys.path.append(_p)

import numpy as np
import ml_dtypes

import concourse.bass as bass
from concourse import bacc
import concourse.mybir as mybir
import concourse.tile as tile
from concourse.bass_utils import run_bass_kernel_spmd

BF16 = mybir.dt.bfloat16
F32 = mybir.dt.float32

B, S, D = 2, 2048, 2048
NH, NKV, HD = 32, 8, 64
GROUP = NH // NKV          # 4 q heads per kv head
NCORES = 8
CPB = NCORES // B          # 4 cores per batch
HPC = NH // CPB            # 8 q heads per core
KVPC = NKV // CPB          # 2 kv heads per core
QW = HPC * HD              # 512 projected q cols per core
KW = KVPC * HD             # 128 projected kv cols per core

NT = S // 128              # 16 seq tiles of 128
NBL = S // 512             # 4 seq blocks of 512
KT = D // 128              # 16 contraction tiles
QF = QW // 128             # 4 row-tiles of Q^T

_nc_cache = None
_DEBUG = False


def _build():
    nc = bacc.Bacc()
    xT = nc.dram_tensor("xT", [D, S], BF16, kind="ExternalInput")
    wq = nc.dram_tensor("wq", [D, QW], BF16, kind="ExternalInput")
    wk = nc.dram_tensor("wk", [D, KW], BF16, kind="ExternalInput")
    wv = nc.dram_tensor("wv", [D, KW], BF16, kind="ExternalInput")
    wo = nc.dram_tensor("wo", [QW, D], BF16, kind="ExternalInput")
    cos2 = nc.dram_tensor("cos2", [128, S], BF16, kind="ExternalInput")
    sinm = nc.dram_tensor("sinm", [128, S], BF16, kind="ExternalInput")
    msk = nc.dram_tensor("msk", [128, 128], BF16, kind="ExternalInput")
    out = nc.dram_tensor("out", [S, D], BF16, kind="ExternalOutput")
    if _DEBUG:
        qdbg = nc.dram_tensor("qdbg", [QF, 128, S], BF16, kind="ExternalOutput")
        kdbg = nc.dram_tensor("kdbg", [128, S], BF16, kind="ExternalOutput")
        vdbg = nc.dram_tensor("vdbg", [KVPC, 128, NT * 65], BF16, kind="ExternalOutput")
        cdbg = nc.dram_tensor("cdbg", [QF, 128, S], BF16, kind="ExternalOutput")

    Exp = mybir.ActivationFunctionType.Exp

    with tile.TileContext(nc) as tc:
        with (
            tc.tile_pool(name="persist", bufs=1) as pp,
            tc.tile_pool(name="psum", bufs=1, space="PSUM") as psp,
            tc.tile_pool(name="attn", bufs=4) as ap,
            tc.tile_pool(name="smal", bufs=1) as sp,
            tc.tile_pool(name="ostg", bufs=2) as op,
        ):
            # ---- persistent tiles ----
            Qb = [pp.tile([128, S], BF16, name=f"qb{f}", tag=f"qb{f}") for f in range(QF)]
            Kb = pp.tile([128, S], BF16, name="kb", tag="kb")
            Vaug = [pp.tile([128, NT, 65], BF16, name=f"vaug{k}", tag=f"vaug{k}") for k in range(KVPC)]
            ctxT = [pp.tile([128, S], BF16, name=f"ctxt{f}", tag=f"ctxt{f}") for f in range(QF)]
            trit = pp.tile([128, 128], BF16, name="trit", tag="trit")

            for k in range(KVPC):
                nc.gpsimd.memset(Vaug[k][:, :, 64:65], 1.0)

            def st_tile():
                return psp.tile([128, 512], F32, name="st", tag="st",
                                bufs=2, padded_shape=[128, 1024])

            def st2_tile():
                return psp.tile([128, 1024], F32, name="st2", tag="st",
                                bufs=2)

            def cp_tile(i):
                return psp.tile([128, 1024], F32, name=f"cp{i}",
                                tag=f"cp{i % 2}", bufs=1)

            # ---- attention block (1024-query half ib2 of head hl) ----
            def attn_block(hl, ib2, inject_map=None):
                kv = hl // GROUP
                fq = hl % GROUP
                qr = kv * 64
                q0 = ib2 * 1024
                jmax = 8 * ib2 + 7
                cp = cp_tile(hl % 2)

                def do_scores(j):
                    jb = j * 128
                    c0 = max(0, jb - q0)
                    st = st2_tile()
                    for lo in (0, 512):
                        hi = lo + 512
                        if hi <= c0:
                            continue
                        l0 = max(lo, c0)
                        nc.tensor.matmul(
                            st[:, l0:hi],
                            Kb[kv * 64:(kv + 1) * 64, jb:jb + 128],
                            Qb[fq][qr:qr + 64, q0 + l0:q0 + hi],
                            start=True, stop=True)
                    pt = ap.tile([128, 1024], BF16, name="pt", tag="pt")
                    nc.scalar.activation(pt[:, c0:1024], st[:, c0:1024],
                                         Exp, scale=0.125)
                    if 0 <= jb - q0 < 1024:
                        # diagonal tile: only the 128-wide band at the
                        # causal boundary needs the triangle mask
                        nc.vector.tensor_mul(pt[:, c0:c0 + 128],
                                             pt[:, c0:c0 + 128],
                                             trit[:])
                    return (j, c0, pt)

                def do_ctx(item):
                    (j, c0, pt) = item
                    for lo in (0, 512):
                        hi = lo + 512
                        if hi <= c0:
                            continue
                        l0 = max(lo, c0)
                        nc.tensor.matmul(
                            cp[0:65, l0:hi], Vaug[kv][:, j, :],
                            pt[:, l0:hi],
                            start=(j == 0), stop=(j == jmax),
                            skip_group_check=True)
                    if j == jmax:
                        norm_ctx()

                def norm_ctx():
                    # denominators sit in row 64 of cp; normalize the 64 ctx
                    # rows into ctxT, freeing the bank. (denom must bounce
                    # via SBUF: custom-DVE ops read garbage from PSUM on hw)
                    dn = sp.tile([1, 1024], F32, name="dn", tag="dn")
                    nc.vector.tensor_copy(dn[0:1, :], cp[64:65, :])
                    rc = sp.tile([1, 1024], F32, name="rc", tag="rc")
                    nc.vector.reciprocal_approx_fast(rc[0:1, :], dn[0:1, :])
                    bc = sp.tile([64, 1024], F32, name="bc", tag="bc")
                    nc.gpsimd.partition_broadcast(bc[0:64, :], rc[0:1, :])
                    nc.vector.tensor_mul(
                        ctxT[fq][qr:qr + 64, q0:q0 + 1024],
                        cp[0:64, :], bc[0:64, :])

                prev = None
                for j in range(jmax + 1):
                    cur = do_scores(j)
                    if prev is not None:
                        do_ctx(prev)
                    if inject_map is not None and j in inject_map:
                        inject_map[j]()
                    prev = cur
                do_ctx(prev)

            # ==== phase 1+2: load x/weights, projections + rope ====
            # (the projection tail is interleaved into attention 3a below)
            with (
                tc.tile_pool(name="proj", bufs=1) as jp,
                tc.tile_pool(name="rope", bufs=2) as rp,
            ):
                xb = jp.tile([128, KT, S], BF16, name="xb", tag="xb")
                wqb = jp.tile([128, KT, QW], BF16, name="wqb", tag="wqb")
                wkb = jp.tile([128, KT, KW], BF16, name="wkb", tag="wkb")
                wvb = jp.tile([128, KT, KW], BF16, name="wvb", tag="wvb")
                cos2t = jp.tile([128, S], BF16, name="cos2t", tag="cos2t")
                sinmt = jp.tile([128, S], BF16, name="sinmt", tag="sinmt")

                # single rearranged DMAs, ordered to match compute
                nc.sync.dma_start(
                    wkb[:], wk[:, :].rearrange("(k p) w -> p k w", p=128))
                nc.sync.dma_start(
                    xb[:, :, 0:512],
                    xT[:, 0:512].rearrange("(k p) c -> p k c", p=128))
                nc.sync.dma_start(
                    wvb[:], wv[:, :].rearrange("(k p) w -> p k w", p=128))
                nc.sync.dma_start(
                    xb[:, :, 512:1024],
                    xT[:, 512:1024].rearrange("(k p) c -> p k c", p=128))
                nc.sync.dma_start(cos2t[:], cos2[:, :])
                nc.sync.dma_start(sinmt[:], sinm[:, :])
                nc.sync.dma_start(
                    wqb[:], wq[:, :].rearrange("(k p) w -> p k w", p=128))
                nc.scalar.dma_start(
                    xb[:, :, 1024:1536],
                    xT[:, 1024:1536].rearrange("(k p) c -> p k c", p=128))
                nc.scalar.dma_start(
                    xb[:, :, 1536:S],
                    xT[:, 1536:S].rearrange("(k p) c -> p k c", p=128))
                nc.sync.dma_start(trit[:], msk[:, :])

                def rope_store(ps, dst, tcol):
                    # ps: psum [128, 512] f32 holding raw Q^T/K^T rows.
                    # dst[:, tcol:tcol+512] <- rope(ps) in bf16.
                    qf = rp.tile([128, 512], F32, name="ropecp", tag="ropecp")
                    nc.scalar.copy(qf[:], ps[:])
                    rot = rp.tile([128, 512], F32, name="roperot", tag="roperot")
                    for base in (0, 64):
                        nc.gpsimd.dma_start(rot[base:base + 32, :],
                                            qf[base + 32:base + 64, :])
                        nc.gpsimd.dma_start(rot[base + 32:base + 64, :],
                                            qf[base:base + 32, :])
                    a = rp.tile([128, 512], F32, name="ropea", tag="ropea")
                    b = rp.tile([128, 512], F32, name="ropeb", tag="ropeb")
                    nc.vector.tensor_mul(a[:], qf[:], cos2t[:, tcol:tcol + 512])
                    nc.vector.tensor_mul(b[:], rot[:], sinmt[:, tcol:tcol + 512])
                    nc.vector.tensor_add(dst[:, tcol:tcol + 512], a[:], b[:])

                def proj_k(t):
                    ps = st_tile()
                    for k in range(KT):
                        nc.tensor.matmul(
                            ps[:], wkb[:, k, :], xb[:, k, t * 512:(t + 1) * 512],
                            start=(k == 0), stop=(k == KT - 1))
                    rope_store(ps, Kb, t * 512)

                def proj_q(f, t):
                    ps = st_tile()
                    for k in range(KT):
                        nc.tensor.matmul(
                            ps[:], wqb[:, k, f * 128:(f + 1) * 128],
                            xb[:, k, t * 512:(t + 1) * 512],
                            start=(k == 0), stop=(k == KT - 1))
                    rope_store(ps, Qb[f], t * 512)

                def proj_v(tt):
                    # V directly in [token, feature] layout: x^T tile is the
                    # stationary operand, wv streams. out [128 tok, 128 feat].
                    ps = psp.tile([128, 128], F32, name="vp", tag="st", bufs=2,
                                  padded_shape=[128, 1024])
                    for k in range(KT):
                        nc.tensor.matmul(
                            ps[:], xb[:, k, tt * 128:(tt + 1) * 128],
                            wvb[:, k, 0:KW],
                            start=(k == 0), stop=(k == KT - 1))
                    for kv in range(KVPC):
                        nc.vector.tensor_copy(Vaug[kv][:, tt, 0:64],
                                              ps[:, kv * 64:(kv + 1) * 64])

                # everything attention 3a needs (keys/queries 0:1024):
                proj_k(0)
                for tt in range(0, 4):
                    proj_v(tt)
                proj_k(1)
                for tt in range(4, 8):
                    proj_v(tt)
                for f in range(QF):
                    proj_q(f, 0)
                for f in range(QF):
                    proj_q(f, 1)

                # remaining projection work, interleaved between 3a heads
                units = ([lambda t=t: proj_k(t) for t in (2, 3)]
                         + [lambda tt=tt: proj_v(tt) for tt in range(8, 16)]
                         + [lambda f=f, t=t: proj_q(f, t)
                            for t in (2, 3) for f in range(QF)])

                # ==== phase 3a: first query half, proj tail interleaved
                # (one unit injected mid-block, one between blocks) ====
                ui = [0]

                def next_unit():
                    if ui[0] < len(units):
                        units[ui[0]]()
                        ui[0] += 1

                for hl in range(HPC):
                    attn_block(hl, 0, inject_map={2: next_unit, 5: next_unit})
                    next_unit()
                while ui[0] < len(units):
                    next_unit()

            # ==== phase 3b + 4: second half + output projection ====
            with tc.tile_pool(name="wout", bufs=1) as wp:
                wot = [wp.tile([128, D], BF16, name=f"wot{c}", tag=f"wot{c}")
                       for c in range(QF)]
                for c in range(QF):
                    nc.sync.dma_start(wot[c][:], wo[c * 128:(c + 1) * 128, :])

                def outproj_tile(t, tagid):
                    # one full output token-tile: 16 dense matmuls with no
                    # cross-engine deps — a long wait-free PE run that fires
                    # the HAM warm-up when injected inside attention.
                    ob = op.tile([128, D], BF16, name="ob", tag="ob")
                    ps2 = psp.tile([128, 2, 512], F32, name="ops",
                                   tag=f"cp{tagid}", bufs=1)
                    for o in range(NBL):
                        h = o % 2
                        for c in range(QF):
                            nc.tensor.matmul(
                                ps2[:, h, :],
                                ctxT[c][:, t * 128:(t + 1) * 128],
                                wot[c][:, o * 512:(o + 1) * 512],
                                start=(c == 0), stop=(c == QF - 1))
                        nc.vector.tensor_copy(ob[:, o * 512:(o + 1) * 512],
                                              ps2[:, h, :])
                    nc.gpsimd.dma_start(out[t * 128:(t + 1) * 128, :], ob[:])

                def outproj_half(t, half, tagid, ob):
                    # 8 dense matmuls (half an output token-tile): wait-free
                    # PE run that keeps the HAM clock warm inside attention
                    ps2 = psp.tile([128, 2, 512], F32, name="ops",
                                   tag=f"cp{tagid}", bufs=1)
                    for o in (2 * half, 2 * half + 1):
                        h = o % 2
                        for c in range(QF):
                            nc.tensor.matmul(
                                ps2[:, h, :],
                                ctxT[c][:, t * 128:(t + 1) * 128],
                                wot[c][:, o * 512:(o + 1) * 512],
                                start=(c == 0), stop=(c == QF - 1))
                        nc.vector.tensor_copy(ob[:, o * 512:(o + 1) * 512],
                                              ps2[:, h, :])
                    if half == 1:
                        nc.gpsimd.dma_start(out[t * 128:(t + 1) * 128, :],
                                            ob[:])

                for hl in range(HPC):
                    ob = op.tile([128, D], BF16, name="ob", tag="ob")
                    attn_block(hl, 1, inject_map={
                        5: (lambda t=hl, g=(hl + 1) % 2, o=ob:
                            outproj_half(t, 0, g, o)),
                        11: (lambda t=hl, g=(hl + 1) % 2, o=ob:
                             outproj_half(t, 1, g, o))})

                # ==== phase 4: remaining out token-tiles, stationary-reuse
                # loop order (ldweights once per contraction tile) ====
                for t in range(8, NT):
                    ob = op.tile([128, D], BF16, name="ob", tag="ob")
                    pst = [psp.tile([128, 512], F32, name="ops4", tag=tag,
                                    bufs=bf, padded_shape=[128, 1024])
                           for tag, bf in (("st", 2), ("st", 2),
                                           ("cp0", 1), ("cp1", 1))]
                    for c in range(QF):
                        for o in range(NBL):
                            nc.tensor.matmul(
                                pst[o][:],
                                ctxT[c][:, t * 128:(t + 1) * 128],
                                wot[c][:, o * 512:(o + 1) * 512],
                                start=(c == 0), stop=(c == QF - 1))
                    for o in range(NBL):
                        nc.vector.tensor_copy(ob[:, o * 512:(o + 1) * 512],
                                              pst[o][:])
                    nc.gpsimd.dma_start(out[t * 128:(t + 1) * 128, :], ob[:])

                if _DEBUG:
                    for f in range(QF):
                        nc.sync.dma_start(qdbg[f, :, :], Qb[f][:])
                        nc.sync.dma_start(cdbg[f, :, :], ctxT[f][:])
                    nc.sync.dma_start(kdbg[:, :], Kb[:])
                    for kv in range(KVPC):
                        nc.sync.dma_start(vdbg[kv, :, :],
                                          Vaug[kv][:, :, :])

    nc.finalize()
    return nc


def _get_nc():
    global _nc_cache
    if _nc_cache is None:
        _nc_cache = _build()
    return _nc_cache


def _prep_inputs(x, cos, sin, Wq, Wk, Wv, Wo):
    bf = ml_dtypes.bfloat16
    cosT = np.ascontiguousarray(cos.T.astype(np.float32))          # [64, S]
    sinT = sin.T.astype(np.float32)
    sinm64 = np.concatenate([-sinT[:32], sinT[32:]], axis=0)       # [64, S]
    cos2 = np.ascontiguousarray(np.concatenate([cosT, cosT], 0)).astype(bf)
    sinm = np.ascontiguousarray(np.concatenate([sinm64, sinm64], 0)).astype(bf)
    msk = (np.arange(128)[:, None] <= np.arange(128)[None, :]).astype(bf)

    # head permutation: Q^T tile f holds local heads (f, f+4) so that the
    # kv0/kv1 row base of K matches the q row base (PE base-partition rule)
    perm = [0, 4, 1, 5, 2, 6, 3, 7]
    colperm = np.concatenate(
        [np.arange(HD) + p * HD for p in perm])          # [QW]
    in_maps = []
    for c in range(NCORES):
        b, g = c // CPB, c % CPB
        xTb = np.ascontiguousarray(x[b].T.astype(bf))
        wq_g = Wq[:, g * QW:(g + 1) * QW][:, colperm]
        wo_g = Wo[g * QW:(g + 1) * QW, :][colperm, :]
        in_maps.append({
            "xT": xTb,
            "wq": np.ascontiguousarray(wq_g.astype(bf)),
            "wk": np.ascontiguousarray(Wk[:, g * KW:(g + 1) * KW].astype(bf)),
            "wv": np.ascontiguousarray(Wv[:, g * KW:(g + 1) * KW].astype(bf)),
            "wo": np.ascontiguousarray(wo_g.astype(bf)),
            "cos2": cos2,
            "sinm": sinm,
            "msk": msk,
        })
    return in_maps


def kernel(x, mask, cos, sin, Wq, Wk, Wv, Wo, _trace=False, **kw):
    x = np.asarray(x, dtype=np.float32)
    in_maps = _prep_inputs(x, np.asarray(cos), np.asarray(sin),
                           np.asarray(Wq), np.asarray(Wk),
                           np.asarray(Wv), np.asarray(Wo))
    nc = _get_nc()
    res = run_bass_kernel_spmd(nc, in_maps, core_ids=list(range(NCORES)),
                               trace=_trace, **kw)
    parts = [np.asarray(r["out"], dtype=np.float32) for r in res.results]
    full = np.stack([
        sum(parts[b * CPB + g] for g in range(CPB)) for b in range(B)
    ]).astype(np.float32)
    if _trace:
        kernel.last_result = res
    return full


# revision 22
# speedup vs baseline: 1.2211x; 1.0453x over previous
"""GQA attention kernel for Trainium2, 8 NeuronCores.

Sharding: core c -> (batch = c // 4, head-group g = c % 4).
Each core handles one batch and 8 contiguous Q heads (= 2 KV heads),
computes its slice of Q/K/V projections, RoPE, causal attention, and a
partial output projection (rows g*512:(g+1)*512 of Wo). Host sums the 4
partials per batch.

Layout: projections are computed transposed (W^T-stationary matmuls on
x^T) producing Q^T/K^T ([feature, token]), the layout the scores matmul
needs. V is projected directly in [token, feature] layout (x^T tile as
the stationary operand) so no PE transposes are needed. Scores are
computed transposed (S^T[j,i], keys on partitions) in 1024-query
blocks so the context matmul consumes exp(S^T) with V as the stationary
operand; a ones column appended to V yields softmax denominators in the
same PSUM accumulation.

Scheduling is built around the PE HAM clock gate (PE runs at 1.2 GHz
until it sees a ~3.4us window of gapless activity, then 2.4 GHz):
 - attention is software-pipelined per head with a one-j-step lag
   between scores and the context matmul consuming its exp;
 - the projection tail is interleaved between attention heads of the
   first query half, and output-projection token-tiles (16 dense
   back-to-back matmuls each) are injected between heads of the second
   half, so the PE always has long wait-free runs that keep the clock
   warm;
 - bulk inputs are loaded with single rearranged DMAs (dma_start issue
   costs ~0.6us each on the sync queue), and small rope/output DMAs go
   through the gpsimd queue.
"""

import sys
import math

for _p in ("/opt/trn_rl_repo",):
    if _p not in sys.path:
        sys.path.append(_p)

import numpy as np
import ml_dtypes

import concourse.bass as bass
from concourse import bacc
import concourse.mybir as mybir
import concourse.tile as tile
from concourse.bass_utils import run_bass_kernel_spmd

BF16 = mybir.dt.bfloat16
F32 = mybir.dt.float32

B, S, D = 2, 2048, 2048
NH, NKV, HD = 32, 8, 64
GROUP = NH // NKV          # 4 q heads per kv head
NCORES = 8
CPB = NCORES // B          # 4 cores per batch
HPC = NH // CPB            # 8 q heads per core
KVPC = NKV // CPB          # 2 kv heads per core
QW = HPC * HD              # 512 projected q cols per core
KW = KVPC * HD             # 128 projected kv cols per core

NT = S // 128              # 16 seq tiles of 128
NBL = S // 512             # 4 seq blocks of 512
KT = D // 128              # 16 contraction tiles
QF = QW // 128             # 4 row-tiles of Q^T

_nc_cache = None
_DEBUG = False


def _build():
    nc = bacc.Bacc()
    xT = nc.dram_tensor("xT", [D, S], BF16, kind="ExternalInput")
    wq = nc.dram_tensor("wq", [D, QW], BF16, kind="ExternalInput")
    wk = nc.dram_tensor("wk", [D, KW], BF16, kind="ExternalInput")
    wv = nc.dram_tensor("wv", [D, KW], BF16, kind="ExternalInput")
    wo = nc.dram_tensor("wo", [QW, D], BF16, kind="ExternalInput")
    cos2 = nc.dram_tensor("cos2", [128, S], BF16, kind="ExternalInput")
    sinm = nc.dram_tensor("sinm", [128, S], BF16, kind="ExternalInput")
    msk = nc.dram_tensor("msk", [128, 128], BF16, kind="ExternalInput")
    out = nc.dram_tensor("out", [S, D], BF16, kind="ExternalOutput")
    if _DEBUG:
        qdbg = nc.dram_tensor("qdbg", [QF, 128, S], BF16, kind="ExternalOutput")
        kdbg = nc.dram_tensor("kdbg", [128, S], BF16, kind="ExternalOutput")
        vdbg = nc.dram_tensor("vdbg", [KVPC, 128, NT * 65], BF16, kind="ExternalOutput")
        cdbg = nc.dram_tensor("cdbg", [QF, 128, S], BF16, kind="ExternalOutput")

    Exp = mybir.ActivationFunctionType.Exp

    with tile.TileContext(nc) as tc:
        with (
            tc.tile_pool(name="persist", bufs=1) as pp,
            tc.tile_pool(name="psum", bufs=1, space="PSUM") as psp,
            tc.tile_pool(name="attn", bufs=4) as ap,
            tc.tile_pool(name="smal", bufs=1) as sp,
            tc.tile_pool(name="ostg", bufs=2) as op,
        ):
            # ---- persistent tiles ----
            Qb = [pp.tile([128, S], BF16, name=f"qb{f}", tag=f"qb{f}") for f in range(QF)]
            Kb = pp.tile([128, S], BF16, name="kb", tag="kb")
            Vaug = [pp.tile([128, NT, 65], BF16, name=f"vaug{k}", tag=f"vaug{k}") for k in range(KVPC)]
            ctxT = [pp.tile([128, S], BF16, name=f"ctxt{f}", tag=f"ctxt{f}") for f in range(QF)]
            trit = pp.tile([128, 128], BF16, name="trit", tag="trit")

            for k in range(KVPC):
                nc.gpsimd.memset(Vaug[k][:, :, 64:65], 1.0)

            def st_tile():
                return psp.tile([128, 512], F32, name="st", tag="st",
                                bufs=2, padded_shape=[128, 1024])

            def st2_tile():
                return psp.tile([128, 1024], F32, name="st2", tag="st",
                                bufs=2)

            def cp_tile(i):
                return psp.tile([128, 1024], F32, name=f"cp{i}",
                                tag=f"cp{i % 2}", bufs=1)

            from collections import deque
            defer = deque()

            # ---- attention block (1024-query half ib2 of head hl) ----
            def attn_block(hl, ib2, inject_map=None):
                kv = hl // GROUP
                fq = hl % GROUP
                qr = kv * 64
                q0 = ib2 * 1024
                jmax = 8 * ib2 + 7
                cp = cp_tile(hl % 2)

                def do_scores(j):
                    jb = j * 128
                    c0 = max(0, jb - q0)
                    st = st2_tile()
                    for lo in (0, 512):
                        hi = lo + 512
                        if hi <= c0:
                            continue
                        l0 = max(lo, c0)
                        nc.tensor.matmul(
                            st[:, l0:hi],
                            Kb[kv * 64:(kv + 1) * 64, jb:jb + 128],
                            Qb[fq][qr:qr + 64, q0 + l0:q0 + hi],
                            start=True, stop=True)
                    pt = ap.tile([128, 1024], BF16, name="pt", tag="pt")
                    nc.scalar.activation(pt[:, c0:1024], st[:, c0:1024],
                                         Exp, scale=0.125)
                    if 0 <= jb - q0 < 1024:
                        # diagonal tile: only the 128-wide band at the
                        # causal boundary needs the triangle mask
                        nc.vector.tensor_mul(pt[:, c0:c0 + 128],
                                             pt[:, c0:c0 + 128],
                                             trit[:])
                    return (j, c0, pt)

                def do_ctx(item):
                    (j, c0, pt) = item
                    for lo in (0, 512):
                        hi = lo + 512
                        if hi <= c0:
                            continue
                        l0 = max(lo, c0)
                        nc.tensor.matmul(
                            cp[0:65, l0:hi], Vaug[kv][:, j, :],
                            pt[:, l0:hi],
                            start=(j == 0), stop=(j == jmax),
                            skip_group_check=True)
                    if j == jmax:
                        norm_ctx()

                def norm_ctx():
                    # denominators sit in row 64 of cp; normalize the 64 ctx
                    # rows into ctxT, freeing the bank. Only the denominator
                    # copy runs here; the reciprocal/broadcast/multiply are
                    # deferred into the next block's j-loop so the DVE FIFO
                    # never parks a ~3.3us chain in front of the mask
                    # multiplies. (denom must bounce via SBUF: custom-DVE
                    # ops read garbage from PSUM on hw)
                    dn = sp.tile([1, 1024], F32, name="dn", tag="dn")
                    nc.vector.tensor_copy(dn[0:1, :], cp[64:65, :])

                    def d_recip():
                        rc = sp.tile([1, 1024], F32, name="rc", tag="rc")
                        nc.vector.reciprocal_approx_fast(rc[0:1, :],
                                                         dn[0:1, :])
                        bc = sp.tile([64, 1024], F32, name="bc", tag="bc")
                        nc.gpsimd.partition_broadcast(bc[0:64, :], rc[0:1, :])
                        norm_ctx.bc = bc

                    def d_mul():
                        nc.vector.tensor_mul(
                            ctxT[fq][qr:qr + 64, q0:q0 + 1024],
                            cp[0:64, :], norm_ctx.bc[0:64, :])
                    defer.append(d_recip)
                    defer.append(d_mul)

                prev = None
                for j in range(jmax + 1):
                    cur = do_scores(j)
                    if prev is not None:
                        do_ctx(prev)
                    if j in (3, 4) and defer:
                        defer.popleft()()
                    if inject_map is not None and j in inject_map:
                        inject_map[j]()
                    prev = cur
                do_ctx(prev)

            # ==== phase 1+2: load x/weights, projections + rope ====
            # (the projection tail is interleaved into attention 3a below)
            with (
                tc.tile_pool(name="proj", bufs=1) as jp,
                tc.tile_pool(name="rope", bufs=2) as rp,
            ):
                xb = jp.tile([128, KT, S], BF16, name="xb", tag="xb")
                wqb = jp.tile([128, KT, QW], BF16, name="wqb", tag="wqb")
                wkb = jp.tile([128, KT, KW], BF16, name="wkb", tag="wkb")
                wvb = jp.tile([128, KT, KW], BF16, name="wvb", tag="wvb")
                cos2t = jp.tile([128, S], BF16, name="cos2t", tag="cos2t")
                sinmt = jp.tile([128, S], BF16, name="sinmt", tag="sinmt")

                # single rearranged DMAs, ordered to match compute
                nc.sync.dma_start(
                    wkb[:], wk[:, :].rearrange("(k p) w -> p k w", p=128))
                nc.sync.dma_start(
                    xb[:, 0:8, 0:512],
                    xT[0:1024, 0:512].rearrange("(k p) c -> p k c", p=128))
                nc.sync.dma_start(
                    xb[:, 8:KT, 0:512],
                    xT[1024:D, 0:512].rearrange("(k p) c -> p k c", p=128))
                nc.sync.dma_start(
                    wvb[:], wv[:, :].rearrange("(k p) w -> p k w", p=128))
                nc.sync.dma_start(
                    xb[:, :, 512:1024],
                    xT[:, 512:1024].rearrange("(k p) c -> p k c", p=128))
                nc.sync.dma_start(cos2t[:], cos2[:, :])
                nc.sync.dma_start(sinmt[:], sinm[:, :])
                nc.sync.dma_start(
                    wqb[:], wq[:, :].rearrange("(k p) w -> p k w", p=128))
                nc.sync.dma_start(
                    xb[:, :, 1024:1536],
                    xT[:, 1024:1536].rearrange("(k p) c -> p k c", p=128))
                nc.sync.dma_start(
                    xb[:, :, 1536:S],
                    xT[:, 1536:S].rearrange("(k p) c -> p k c", p=128))
                nc.sync.dma_start(trit[:], msk[:, :])

                def rope_store(ps, dst, tcol):
                    # ps: psum [128, 512] f32 holding raw Q^T/K^T rows.
                    # dst[:, tcol:tcol+512] <- rope(ps) in bf16.
                    qf = rp.tile([128, 512], F32, name="ropecp", tag="ropecp")
                    nc.scalar.copy(qf[:], ps[:])
                    rot = rp.tile([128, 512], F32, name="roperot", tag="roperot")
                    for base in (0, 64):
                        nc.gpsimd.dma_start(rot[base:base + 32, :],
                                            qf[base + 32:base + 64, :])
                        nc.gpsimd.dma_start(rot[base + 32:base + 64, :],
                                            qf[base:base + 32, :])
                    a = rp.tile([128, 512], F32, name="ropea", tag="ropea")
                    b = rp.tile([128, 512], F32, name="ropeb", tag="ropeb")
                    nc.vector.tensor_mul(a[:], qf[:], cos2t[:, tcol:tcol + 512])
                    nc.vector.tensor_mul(b[:], rot[:], sinmt[:, tcol:tcol + 512])
                    nc.vector.tensor_add(dst[:, tcol:tcol + 512], a[:], b[:])

                def proj_k(t):
                    ps = st_tile()
                    for k in range(KT):
                        nc.tensor.matmul(
                            ps[:], wkb[:, k, :], xb[:, k, t * 512:(t + 1) * 512],
                            start=(k == 0), stop=(k == KT - 1))
                    rope_store(ps, Kb, t * 512)

                def proj_q(f, t):
                    ps = st_tile()
                    for k in range(KT):
                        nc.tensor.matmul(
                            ps[:], wqb[:, k, f * 128:(f + 1) * 128],
                            xb[:, k, t * 512:(t + 1) * 512],
                            start=(k == 0), stop=(k == KT - 1))
                    rope_store(ps, Qb[f], t * 512)

                def proj_v(tt):
                    # V directly in [token, feature] layout: x^T tile is the
                    # stationary operand, wv streams. out [128 tok, 128 feat].
                    ps = psp.tile([128, 128], F32, name="vp", tag="st", bufs=2,
                                  padded_shape=[128, 1024])
                    for k in range(KT):
                        nc.tensor.matmul(
                            ps[:], xb[:, k, tt * 128:(tt + 1) * 128],
                            wvb[:, k, 0:KW],
                            start=(k == 0), stop=(k == KT - 1))
                    for kv in range(KVPC):
                        nc.vector.tensor_copy(Vaug[kv][:, tt, 0:64],
                                              ps[:, kv * 64:(kv + 1) * 64])

                # everything attention 3a needs (keys/queries 0:1024):
                proj_k(0)
                for tt in range(0, 4):
                    proj_v(tt)
                proj_k(1)
                for tt in range(4, 8):
                    proj_v(tt)
                for f in range(QF):
                    proj_q(f, 0)
                for f in range(QF):
                    proj_q(f, 1)

                # remaining projection work, interleaved between 3a heads
                units = ([lambda t=t: proj_k(t) for t in (2, 3)]
                         + [lambda tt=tt: proj_v(tt) for tt in range(8, 16)]
                         + [lambda f=f, t=t: proj_q(f, t)
                            for t in (2, 3) for f in range(QF)])

                # ==== phase 3a: first query half, proj tail interleaved
                # (one unit injected mid-block, one between blocks) ====
                ui = [0]

                def next_unit():
                    if ui[0] < len(units):
                        units[ui[0]]()
                        ui[0] += 1

                for hl in range(HPC):
                    attn_block(hl, 0, inject_map={2: next_unit, 5: next_unit})
                    next_unit()
                while ui[0] < len(units):
                    next_unit()

            # ==== phase 3b + 4: second half + output projection ====
            with tc.tile_pool(name="wout", bufs=1) as wp:
                wot = [wp.tile([128, D], BF16, name=f"wot{c}", tag=f"wot{c}")
                       for c in range(QF)]
                for c in range(QF):
                    nc.sync.dma_start(wot[c][:], wo[c * 128:(c + 1) * 128, :])

                def outproj_tile(t, tagid):
                    # one full output token-tile: 16 dense matmuls with no
                    # cross-engine deps — a long wait-free PE run that fires
                    # the HAM warm-up when injected inside attention.
                    ob = op.tile([128, D], BF16, name="ob", tag="ob")
                    ps2 = psp.tile([128, 2, 512], F32, name="ops",
                                   tag=f"cp{tagid}", bufs=1)
                    for o in range(NBL):
                        h = o % 2
                        for c in range(QF):
                            nc.tensor.matmul(
                                ps2[:, h, :],
                                ctxT[c][:, t * 128:(t + 1) * 128],
                                wot[c][:, o * 512:(o + 1) * 512],
                                start=(c == 0), stop=(c == QF - 1))
                        nc.vector.tensor_copy(ob[:, o * 512:(o + 1) * 512],
                                              ps2[:, h, :])
                    nc.gpsimd.dma_start(out[t * 128:(t + 1) * 128, :], ob[:])

                def outproj_half(t, half, tagid, ob):
                    # 8 dense matmuls (half an output token-tile): wait-free
                    # PE run that keeps the HAM clock warm inside attention
                    ps2 = psp.tile([128, 2, 512], F32, name="ops",
                                   tag=f"cp{tagid}", bufs=1)
                    for o in (2 * half, 2 * half + 1):
                        h = o % 2
                        for c in range(QF):
                            nc.tensor.matmul(
                                ps2[:, h, :],
                                ctxT[c][:, t * 128:(t + 1) * 128],
                                wot[c][:, o * 512:(o + 1) * 512],
                                start=(c == 0), stop=(c == QF - 1))
                        nc.vector.tensor_copy(ob[:, o * 512:(o + 1) * 512],
                                              ps2[:, h, :])
                    if half == 1:
                        nc.gpsimd.dma_start(out[t * 128:(t + 1) * 128, :],
                                            ob[:])

                for hl in range(HPC):
                    ob = op.tile([128, D], BF16, name="ob", tag="ob")
                    attn_block(hl, 1, inject_map={
                        5: (lambda t=hl, g=(hl + 1) % 2, o=ob:
                            outproj_half(t, 0, g, o)),
                        11: (lambda t=hl, g=(hl + 1) % 2, o=ob:
                             outproj_half(t, 1, g, o))})

                while defer:
                    defer.popleft()()

                # ==== phase 4: remaining out token-tiles, stationary-reuse
                # loop order (ldweights once per contraction tile) ====
                for t in range(8, NT):
                    ob = op.tile([128, D], BF16, name="ob", tag="ob")
                    pst = [psp.tile([128, 512], F32, name="ops4", tag=tag,
                                    bufs=bf, padded_shape=[128, 1024])
                           for tag, bf in (("st", 2), ("st", 2),
                                           ("cp0", 1), ("cp1", 1))]
                    for c in range(QF):
                        for o in range(NBL):
                            nc.tensor.matmul(
                                pst[o][:],
                                ctxT[c][:, t * 128:(t + 1) * 128],
                                wot[c][:, o * 512:(o + 1) * 512],
                                start=(c == 0), stop=(c == QF - 1))
                    for o in range(NBL):
                        nc.vector.tensor_copy(ob[:, o * 512:(o + 1) * 512],
                                              pst[o][:])
                    nc.gpsimd.dma_start(out[t * 128:(t + 1) * 128, :], ob[:])

                if _DEBUG:
                    for f in range(QF):
                        nc.sync.dma_start(qdbg[f, :, :], Qb[f][:])
                        nc.sync.dma_start(cdbg[f, :, :], ctxT[f][:])
                    nc.sync.dma_start(kdbg[:, :], Kb[:])
                    for kv in range(KVPC):
                        nc.sync.dma_start(vdbg[kv, :, :],
                                          Vaug[kv][:, :, :])

    nc.finalize()
    return nc


def _get_nc():
    global _nc_cache
    if _nc_cache is None:
        _nc_cache = _build()
    return _nc_cache


def _prep_inputs(x, cos, sin, Wq, Wk, Wv, Wo):
    bf = ml_dtypes.bfloat16
    cosT = np.ascontiguousarray(cos.T.astype(np.float32))          # [64, S]
    sinT = sin.T.astype(np.float32)
    sinm64 = np.concatenate([-sinT[:32], sinT[32:]], axis=0)       # [64, S]
    cos2 = np.ascontiguousarray(np.concatenate([cosT, cosT], 0)).astype(bf)
    sinm = np.ascontiguousarray(np.concatenate([sinm64, sinm64], 0)).astype(bf)
    msk = (np.arange(128)[:, None] <= np.arange(128)[None, :]).astype(bf)

    # head permutation: Q^T tile f holds local heads (f, f+4) so that the
    # kv0/kv1 row base of K matches the q row base (PE base-partition rule)
    perm = [0, 4, 1, 5, 2, 6, 3, 7]
    colperm = np.concatenate(
        [np.arange(HD) + p * HD for p in perm])          # [QW]
    in_maps = []
    for c in range(NCORES):
        b, g = c // CPB, c % CPB
        xTb = np.ascontiguousarray(x[b].T.astype(bf))
        wq_g = Wq[:, g * QW:(g + 1) * QW][:, colperm]
        wo_g = Wo[g * QW:(g + 1) * QW, :][colperm, :]
        in_maps.append({
            "xT": xTb,
            "wq": np.ascontiguousarray(wq_g.astype(bf)),
            "wk": np.ascontiguousarray(Wk[:, g * KW:(g + 1) * KW].astype(bf)),
            "wv": np.ascontiguousarray(Wv[:, g * KW:(g + 1) * KW].astype(bf)),
            "wo": np.ascontiguousarray(wo_g.astype(bf)),
            "cos2": cos2,
            "sinm": sinm,
            "msk": msk,
        })
    return in_maps


def kernel(x, mask, cos, sin, Wq, Wk, Wv, Wo, _trace=False, **kw):
    x = np.asarray(x, dtype=np.float32)
    in_maps = _prep_inputs(x, np.asarray(cos), np.asarray(sin),
                           np.asarray(Wq), np.asarray(Wk),
                           np.asarray(Wv), np.asarray(Wo))
    nc = _get_nc()
    res = run_bass_kernel_spmd(nc, in_maps, core_ids=list(range(NCORES)),
                               trace=_trace, **kw)
    parts = [np.asarray(r["out"], dtype=np.float32) for r in res.results]
    full = np.stack([
        sum(parts[b * CPB + g] for g in range(CPB)) for b in range(B)
    ]).astype(np.float32)
    if _trace:
        kernel.last_result = res
    return full


# revision 24
# speedup vs baseline: 1.2524x; 1.0256x over previous
"""GQA attention kernel for Trainium2, 8 NeuronCores.

Sharding: core c -> (batch = c // 4, head-group g = c % 4).
Each core handles one batch and 8 contiguous Q heads (= 2 KV heads),
computes its slice of Q/K/V projections, RoPE, causal attention, and a
partial output projection (rows g*512:(g+1)*512 of Wo). Host sums the 4
partials per batch.

Layout: projections are computed transposed (W^T-stationary matmuls on
x^T) producing Q^T/K^T ([feature, token]), the layout the scores matmul
needs. V is projected directly in [token, feature] layout (x^T tile as
the stationary operand) so no PE transposes are needed. Scores are
computed transposed (S^T[j,i], keys on partitions) in 1024-query
blocks so the context matmul consumes exp(S^T) with V as the stationary
operand; a ones column appended to V yields softmax denominators in the
same PSUM accumulation.

Scheduling is built around the PE HAM clock gate (PE runs at 1.2 GHz
until it sees a ~3.4us window of gapless activity, then 2.4 GHz):
 - attention is software-pipelined per head with a one-j-step lag
   between scores and the context matmul consuming its exp;
 - the projection tail is interleaved between attention heads of the
   first query half, and output-projection token-tiles (16 dense
   back-to-back matmuls each) are injected between heads of the second
   half, so the PE always has long wait-free runs that keep the clock
   warm;
 - bulk inputs are loaded with single rearranged DMAs (dma_start issue
   costs ~0.6us each on the sync queue), and small rope/output DMAs go
   through the gpsimd queue.
"""

import sys
import math

for _p in ("/opt/trn_rl_repo",):
    if _p not in sys.path:
        sys.path.append(_p)

import numpy as np
import ml_dtypes

import concourse.bass as bass
from concourse import bacc
import concourse.mybir as mybir
import concourse.tile as tile
from concourse.bass_utils import run_bass_kernel_spmd

BF16 = mybir.dt.bfloat16
F32 = mybir.dt.float32

B, S, D = 2, 2048, 2048
NH, NKV, HD = 32, 8, 64
GROUP = NH // NKV          # 4 q heads per kv head
NCORES = 8
CPB = NCORES // B          # 4 cores per batch
HPC = NH // CPB            # 8 q heads per core
KVPC = NKV // CPB          # 2 kv heads per core
QW = HPC * HD              # 512 projected q cols per core
KW = KVPC * HD             # 128 projected kv cols per core

NT = S // 128              # 16 seq tiles of 128
NBL = S // 512             # 4 seq blocks of 512
KT = D // 128              # 16 contraction tiles
QF = QW // 128             # 4 row-tiles of Q^T

_nc_cache = None
_DEBUG = False


def _build():
    nc = bacc.Bacc()
    xT = nc.dram_tensor("xT", [D, S], BF16, kind="ExternalInput")
    wq = nc.dram_tensor("wq", [D, QW], BF16, kind="ExternalInput")
    wk = nc.dram_tensor("wk", [D, KW], BF16, kind="ExternalInput")
    wv = nc.dram_tensor("wv", [D, KW], BF16, kind="ExternalInput")
    wo = nc.dram_tensor("wo", [QW, D], BF16, kind="ExternalInput")
    cos2 = nc.dram_tensor("cos2", [128, S], BF16, kind="ExternalInput")
    sinm = nc.dram_tensor("sinm", [128, S], BF16, kind="ExternalInput")
    msk = nc.dram_tensor("msk", [128, 128], BF16, kind="ExternalInput")
    out = nc.dram_tensor("out", [S, D], BF16, kind="ExternalOutput")
    if _DEBUG:
        qdbg = nc.dram_tensor("qdbg", [QF, 128, S], BF16, kind="ExternalOutput")
        kdbg = nc.dram_tensor("kdbg", [128, S], BF16, kind="ExternalOutput")
        vdbg = nc.dram_tensor("vdbg", [KVPC, 128, NT * 65], BF16, kind="ExternalOutput")
        cdbg = nc.dram_tensor("cdbg", [QF, 128, S], BF16, kind="ExternalOutput")

    Exp = mybir.ActivationFunctionType.Exp

    with tile.TileContext(nc) as tc:
        with (
            tc.tile_pool(name="persist", bufs=1) as pp,
            tc.tile_pool(name="psum", bufs=1, space="PSUM") as psp,
            tc.tile_pool(name="attn", bufs=4) as ap,
            tc.tile_pool(name="smal", bufs=1) as sp,
            tc.tile_pool(name="ostg", bufs=2) as op,
        ):
            # ---- persistent tiles ----
            Qb = [pp.tile([128, S], BF16, name=f"qb{f}", tag=f"qb{f}") for f in range(QF)]
            Kb = pp.tile([128, S], BF16, name="kb", tag="kb")
            Vaug = [pp.tile([128, NT, 65], BF16, name=f"vaug{k}", tag=f"vaug{k}") for k in range(KVPC)]
            ctxT = [pp.tile([128, S], BF16, name=f"ctxt{f}", tag=f"ctxt{f}") for f in range(QF)]
            trit = pp.tile([128, 128], BF16, name="trit", tag="trit")

            for k in range(KVPC):
                nc.gpsimd.memset(Vaug[k][:, :, 64:65], 1.0)

            def st_tile():
                return psp.tile([128, 512], F32, name="st", tag="st",
                                bufs=2, padded_shape=[128, 1024])

            def st2_tile():
                return psp.tile([128, 1024], F32, name="st2", tag="st",
                                bufs=2)

            def cp_tile(i):
                return psp.tile([128, 1024], F32, name=f"cp{i}",
                                tag=f"cp{i % 2}", bufs=1)

            from collections import deque
            defer = deque()

            # ---- attention block (1024-query half ib2 of head hl) ----
            def attn_block(hl, ib2, inject_map=None):
                kv = hl // GROUP
                fq = hl % GROUP
                qr = kv * 64
                q0 = ib2 * 1024
                jmax = 8 * ib2 + 7
                cp = cp_tile(hl % 2)

                def do_scores(j):
                    jb = j * 128
                    c0 = max(0, jb - q0)
                    st = st2_tile()
                    for lo in (0, 512):
                        hi = lo + 512
                        if hi <= c0:
                            continue
                        l0 = max(lo, c0)
                        nc.tensor.matmul(
                            st[:, l0:hi],
                            Kb[kv * 64:(kv + 1) * 64, jb:jb + 128],
                            Qb[fq][qr:qr + 64, q0 + l0:q0 + hi],
                            start=True, stop=True)
                    pt = ap.tile([128, 1024], BF16, name="pt", tag="pt")
                    nc.scalar.activation(pt[:, c0:1024], st[:, c0:1024],
                                         Exp, scale=0.125)
                    if 0 <= jb - q0 < 1024:
                        # diagonal tile: only the 128-wide band at the
                        # causal boundary needs the triangle mask
                        nc.vector.tensor_mul(pt[:, c0:c0 + 128],
                                             pt[:, c0:c0 + 128],
                                             trit[:])
                    return (j, c0, pt)

                def do_ctx(item):
                    (j, c0, pt) = item
                    for lo in (0, 512):
                        hi = lo + 512
                        if hi <= c0:
                            continue
                        l0 = max(lo, c0)
                        nc.tensor.matmul(
                            cp[0:65, l0:hi], Vaug[kv][:, j, :],
                            pt[:, l0:hi],
                            start=(j == 0), stop=(j == jmax),
                            skip_group_check=True)
                    if j == jmax:
                        norm_ctx()

                def norm_ctx():
                    # denominators sit in row 64 of cp; normalize the 64 ctx
                    # rows into ctxT, freeing the bank. Only the denominator
                    # copy runs here; the reciprocal/broadcast/multiply are
                    # deferred into the next block's j-loop so the DVE FIFO
                    # never parks a ~3.3us chain in front of the mask
                    # multiplies. (denom must bounce via SBUF: custom-DVE
                    # ops read garbage from PSUM on hw)
                    dn = sp.tile([1, 1024], F32, name="dn", tag="dn")
                    nc.vector.tensor_copy(dn[0:1, :], cp[64:65, :])

                    def d_recip():
                        rc = sp.tile([1, 1024], F32, name="rc", tag="rc")
                        nc.vector.reciprocal_approx_fast(rc[0:1, :],
                                                         dn[0:1, :])
                        bc = sp.tile([64, 1024], F32, name="bc", tag="bc")
                        nc.gpsimd.partition_broadcast(bc[0:64, :], rc[0:1, :])
                        norm_ctx.bc = bc

                    def d_mul():
                        nc.vector.tensor_mul(
                            ctxT[fq][qr:qr + 64, q0:q0 + 1024],
                            cp[0:64, :], norm_ctx.bc[0:64, :])
                    defer.append(d_recip)
                    defer.append(d_mul)

                prev = None
                for j in range(jmax + 1):
                    cur = do_scores(j)
                    if prev is not None:
                        do_ctx(prev)
                    # deferred norm ops must be emitted before any reuse of
                    # their cp slot (units/injections at j>=3)
                    if j in (1, 2) and defer:
                        defer.popleft()()
                    if inject_map is not None and j in inject_map:
                        inject_map[j]()
                    prev = cur
                do_ctx(prev)

            # ==== phase 1+2: load x/weights, projections + rope ====
            # (the projection tail is interleaved into attention 3a below)
            with (
                tc.tile_pool(name="proj", bufs=1) as jp,
                tc.tile_pool(name="rope", bufs=2) as rp,
            ):
                xb = jp.tile([128, KT, S], BF16, name="xb", tag="xb")
                wqb = jp.tile([128, KT, QW], BF16, name="wqb", tag="wqb")
                wkb = jp.tile([128, KT, KW], BF16, name="wkb", tag="wkb")
                wvb = jp.tile([128, KT, KW], BF16, name="wvb", tag="wvb")
                cos2t = jp.tile([128, S], BF16, name="cos2t", tag="cos2t")
                sinmt = jp.tile([128, S], BF16, name="sinmt", tag="sinmt")

                # single rearranged DMAs, ordered to match compute
                nc.sync.dma_start(
                    wkb[:], wk[:, :].rearrange("(k p) w -> p k w", p=128))
                nc.sync.dma_start(
                    xb[:, 0:8, 0:512],
                    xT[0:1024, 0:512].rearrange("(k p) c -> p k c", p=128))
                nc.sync.dma_start(
                    xb[:, 8:KT, 0:512],
                    xT[1024:D, 0:512].rearrange("(k p) c -> p k c", p=128))
                nc.sync.dma_start(
                    wvb[:], wv[:, :].rearrange("(k p) w -> p k w", p=128))
                nc.sync.dma_start(
                    xb[:, :, 512:1024],
                    xT[:, 512:1024].rearrange("(k p) c -> p k c", p=128))
                nc.sync.dma_start(cos2t[:], cos2[:, :])
                nc.sync.dma_start(sinmt[:], sinm[:, :])
                nc.sync.dma_start(
                    wqb[:], wq[:, :].rearrange("(k p) w -> p k w", p=128))
                nc.sync.dma_start(
                    xb[:, :, 1024:1536],
                    xT[:, 1024:1536].rearrange("(k p) c -> p k c", p=128))
                nc.sync.dma_start(
                    xb[:, :, 1536:S],
                    xT[:, 1536:S].rearrange("(k p) c -> p k c", p=128))
                nc.sync.dma_start(trit[:], msk[:, :])

                def rope_store(ps, dst, tcol):
                    # ps: psum [128, 512] f32 holding raw Q^T/K^T rows.
                    # dst[:, tcol:tcol+512] <- rope(ps) in bf16.
                    qf = rp.tile([128, 512], F32, name="ropecp", tag="ropecp")
                    nc.scalar.copy(qf[:], ps[:])
                    rot = rp.tile([128, 512], F32, name="roperot", tag="roperot")
                    for base in (0, 64):
                        nc.gpsimd.dma_start(rot[base:base + 32, :],
                                            qf[base + 32:base + 64, :])
                        nc.gpsimd.dma_start(rot[base + 32:base + 64, :],
                                            qf[base:base + 32, :])
                    a = rp.tile([128, 512], F32, name="ropea", tag="ropea")
                    b = rp.tile([128, 512], F32, name="ropeb", tag="ropeb")
                    nc.vector.tensor_mul(a[:], qf[:], cos2t[:, tcol:tcol + 512])
                    nc.vector.tensor_mul(b[:], rot[:], sinmt[:, tcol:tcol + 512])
                    nc.vector.tensor_add(dst[:, tcol:tcol + 512], a[:], b[:])

                def proj_k(t, tag="st"):
                    ps = psp.tile([128, 512], F32, name="st", tag=tag,
                                  bufs=(2 if tag == "st" else 1),
                                  padded_shape=[128, 1024])
                    for k in range(KT):
                        nc.tensor.matmul(
                            ps[:], wkb[:, k, :], xb[:, k, t * 512:(t + 1) * 512],
                            start=(k == 0), stop=(k == KT - 1))
                    rope_store(ps, Kb, t * 512)

                def proj_q(f, t, tag="st"):
                    ps = psp.tile([128, 512], F32, name="st", tag=tag,
                                  bufs=(2 if tag == "st" else 1),
                                  padded_shape=[128, 1024])
                    for k in range(KT):
                        nc.tensor.matmul(
                            ps[:], wqb[:, k, f * 128:(f + 1) * 128],
                            xb[:, k, t * 512:(t + 1) * 512],
                            start=(k == 0), stop=(k == KT - 1))
                    rope_store(ps, Qb[f], t * 512)

                def proj_v(tt, tag="st"):
                    # V directly in [token, feature] layout: x^T tile is the
                    # stationary operand, wv streams. out [128 tok, 128 feat].
                    ps = psp.tile([128, 128], F32, name="vp", tag=tag,
                                  bufs=(2 if tag == "st" else 1),
                                  padded_shape=[128, 1024])
                    for k in range(KT):
                        nc.tensor.matmul(
                            ps[:], xb[:, k, tt * 128:(tt + 1) * 128],
                            wvb[:, k, 0:KW],
                            start=(k == 0), stop=(k == KT - 1))
                    for kv in range(KVPC):
                        nc.vector.tensor_copy(Vaug[kv][:, tt, 0:64],
                                              ps[:, kv * 64:(kv + 1) * 64])

                # everything attention 3a needs (keys/queries 0:1024):
                proj_k(0)
                for tt in range(0, 4):
                    proj_v(tt)
                proj_k(1)
                for tt in range(4, 8):
                    proj_v(tt)
                for f in range(QF):
                    proj_q(f, 0)
                for f in range(QF):
                    proj_q(f, 1)

                # remaining projection work, interleaved between 3a heads;
                # the units' PSUM comes from the cp tag the current head is
                # NOT using, so the scores double-buffer never stalls on a
                # unit's rope-copy.
                units = ([lambda tag, t=t: proj_k(t, tag) for t in (2, 3)]
                         + [lambda tag, tt=tt: proj_v(tt, tag)
                            for tt in range(8, 16)]
                         + [lambda tag, f=f, t=t: proj_q(f, t, tag)
                            for t in (2, 3) for f in range(QF)])

                # ==== phase 3a: first query half, proj tail interleaved ====
                ui = [0]

                def next_unit(tag):
                    if ui[0] < len(units):
                        units[ui[0]](tag)
                        ui[0] += 1

                for hl in range(HPC):
                    g = f"cp{(hl + 1) % 2}"
                    attn_block(hl, 0, inject_map={
                        3: (lambda g=g: next_unit(g)),
                        5: (lambda g=g: next_unit(g)),
                        7: (lambda g=g: next_unit(g))})
                while ui[0] < len(units):
                    next_unit("st")

            # ==== phase 3b + 4: second half + output projection ====
            with tc.tile_pool(name="wout", bufs=1) as wp:
                wot = [wp.tile([128, D], BF16, name=f"wot{c}", tag=f"wot{c}")
                       for c in range(QF)]
                for c in range(QF):
                    nc.sync.dma_start(wot[c][:], wo[c * 128:(c + 1) * 128, :])

                def outproj_tile(t, tagid):
                    # one full output token-tile: 16 dense matmuls with no
                    # cross-engine deps — a long wait-free PE run that fires
                    # the HAM warm-up when injected inside attention.
                    ob = op.tile([128, D], BF16, name="ob", tag="ob")
                    ps2 = psp.tile([128, 2, 512], F32, name="ops",
                                   tag=f"cp{tagid}", bufs=1)
                    for o in range(NBL):
                        h = o % 2
                        for c in range(QF):
                            nc.tensor.matmul(
                                ps2[:, h, :],
                                ctxT[c][:, t * 128:(t + 1) * 128],
                                wot[c][:, o * 512:(o + 1) * 512],
                                start=(c == 0), stop=(c == QF - 1))
                        nc.vector.tensor_copy(ob[:, o * 512:(o + 1) * 512],
                                              ps2[:, h, :])
                    nc.gpsimd.dma_start(out[t * 128:(t + 1) * 128, :], ob[:])

                def outproj_half(t, half, tagid, ob):
                    # 8 dense matmuls (half an output token-tile): wait-free
                    # PE run that keeps the HAM clock warm inside attention
                    ps2 = psp.tile([128, 2, 512], F32, name="ops",
                                   tag=f"cp{tagid}", bufs=1)
                    for o in (2 * half, 2 * half + 1):
                        h = o % 2
                        for c in range(QF):
                            nc.tensor.matmul(
                                ps2[:, h, :],
                                ctxT[c][:, t * 128:(t + 1) * 128],
                                wot[c][:, o * 512:(o + 1) * 512],
                                start=(c == 0), stop=(c == QF - 1))
                        nc.vector.tensor_copy(ob[:, o * 512:(o + 1) * 512],
                                              ps2[:, h, :])
                    if half == 1:
                        nc.gpsimd.dma_start(out[t * 128:(t + 1) * 128, :],
                                            ob[:])

                for hl in range(HPC):
                    ob = op.tile([128, D], BF16, name="ob", tag="ob")
                    attn_block(hl, 1, inject_map={
                        5: (lambda t=hl, g=(hl + 1) % 2, o=ob:
                            outproj_half(t, 0, g, o)),
                        11: (lambda t=hl, g=(hl + 1) % 2, o=ob:
                             outproj_half(t, 1, g, o))})

                while defer:
                    defer.popleft()()

                # ==== phase 4: remaining out token-tiles, stationary-reuse
                # loop order (ldweights once per contraction tile) ====
                for t in range(8, NT):
                    ob = op.tile([128, D], BF16, name="ob", tag="ob")
                    pst = [psp.tile([128, 512], F32, name="ops4", tag=tag,
                                    bufs=bf, padded_shape=[128, 1024])
                           for tag, bf in (("st", 2), ("st", 2),
                                           ("cp0", 1), ("cp1", 1))]
                    for c in range(QF):
                        for o in range(NBL):
                            nc.tensor.matmul(
                                pst[o][:],
                                ctxT[c][:, t * 128:(t + 1) * 128],
                                wot[c][:, o * 512:(o + 1) * 512],
                                start=(c == 0), stop=(c == QF - 1))
                    for o in range(NBL):
                        nc.vector.tensor_copy(ob[:, o * 512:(o + 1) * 512],
                                              pst[o][:])
                    nc.gpsimd.dma_start(out[t * 128:(t + 1) * 128, :], ob[:])

                if _DEBUG:
                    for f in range(QF):
                        nc.sync.dma_start(qdbg[f, :, :], Qb[f][:])
                        nc.sync.dma_start(cdbg[f, :, :], ctxT[f][:])
                    nc.sync.dma_start(kdbg[:, :], Kb[:])
                    for kv in range(KVPC):
                        nc.sync.dma_start(vdbg[kv, :, :],
                                          Vaug[kv][:, :, :])

    nc.finalize()
    return nc


def _get_nc():
    global _nc_cache
    if _nc_cache is None:
        _nc_cache = _build()
    return _nc_cache


def _prep_inputs(x, cos, sin, Wq, Wk, Wv, Wo):
    bf = ml_dtypes.bfloat16
    cosT = np.ascontiguousarray(cos.T.astype(np.float32))          # [64, S]
    sinT = sin.T.astype(np.float32)
    sinm64 = np.concatenate([-sinT[:32], sinT[32:]], axis=0)       # [64, S]
    cos2 = np.ascontiguousarray(np.concatenate([cosT, cosT], 0)).astype(bf)
    sinm = np.ascontiguousarray(np.concatenate([sinm64, sinm64], 0)).astype(bf)
    msk = (np.arange(128)[:, None] <= np.arange(128)[None, :]).astype(bf)

    # head permutation: Q^T tile f holds local heads (f, f+4) so that the
    # kv0/kv1 row base of K matches the q row base (PE base-partition rule)
    perm = [0, 4, 1, 5, 2, 6, 3, 7]
    colperm = np.concatenate(
        [np.arange(HD) + p * HD for p in perm])          # [QW]
    in_maps = []
    for c in range(NCORES):
        b, g = c // CPB, c % CPB
        xTb = np.ascontiguousarray(x[b].T.astype(bf))
        wq_g = Wq[:, g * QW:(g + 1) * QW][:, colperm]
        wo_g = Wo[g * QW:(g + 1) * QW, :][colperm, :]
        in_maps.append({
            "xT": xTb,
            "wq": np.ascontiguousarray(wq_g.astype(bf)),
            "wk": np.ascontiguousarray(Wk[:, g * KW:(g + 1) * KW].astype(bf)),
            "wv": np.ascontiguousarray(Wv[:, g * KW:(g + 1) * KW].astype(bf)),
            "wo": np.ascontiguousarray(wo_g.astype(bf)),
            "cos2": cos2,
            "sinm": sinm,
            "msk": msk,
        })
    return in_maps


def kernel(x, mask, cos, sin, Wq, Wk, Wv, Wo, _trace=False, **kw):
    x = np.asarray(x, dtype=np.float32)
    in_maps = _prep_inputs(x, np.asarray(cos), np.asarray(sin),
                           np.asarray(Wq), np.asarray(Wk),
                           np.asarray(Wv), np.asarray(Wo))
    nc = _get_nc()
    res = run_bass_kernel_spmd(nc, in_maps, core_ids=list(range(NCORES)),
                               trace=_trace, **kw)
    parts = [np.asarray(r["out"], dtype=np.float32) for r in res.results]
    full = np.stack([
        sum(parts[b * CPB + g] for g in range(CPB)) for b in range(B)
    ]).astype(np.float32)
    if _trace:
        kernel.last_result = res
    return full
